# revision 31
# baseline (speedup 1.0000x reference)
"""CPR router kernel for Trainium2 (8 NeuronCores, data-parallel over tokens).

Math (matches the jax reference):
    h_n = l2norm(hidden_states, axis=1); p_n = l2norm(proto, axis=1)
    logits = h_n @ p_n.T                      # [T, 64] cosine sims
    w = softmax(logits, axis=1)
    routing_weights, selected_experts = top_k(w, 8)

Device strategy (per core: 2048 tokens, 8 groups x 2 tiles x 128 tokens):
    - Host prep is layout-only: h is split into two limbs (hi = round(h),
      lo = round(h - hi)) and both planes are stored TRANSPOSED in DRAM as
      [2048 d, 2048 t], so the device needs no on-chip transposes and no
      PSUM->SBUF staging copies -- the PE reads d-major slabs from SBUF
      directly.  h == hi + lo to ~2^-18 (bf16x2) or ~2^-14 (f16e5); numpy
      check vs the fp32 reference: 0 / 2 set-mismatched tokens of 16384.
    - proto is tiny: normalized + limb-split + chunk-transposed on host,
      replicated (pt_hi/pt_lo[p, c*64+e] = p_limb[e, c*128+p]).
    - Per group (2 tiles), per chunk c, per tile k -- 4 bf16/f16/fp8-rate
      matmuls (1 cycle/row), all accumulating in PSUM:
        psL[:, k*64:+64] += hi_win.T @ pt_hi[c]     (N=64)
                         += hi_win.T @ pt_lo[c]     (N=64)
                         += lo_win.T @ pt8[c]       (N=64)
        psG[:, k*128:+128] += hi_win.T @ hi_win     (N=128, Gram block)
      The Gram diagonal is sum_d hi[d,t]^2 = ||h_t||^2 (4e-5 rel err that
      only scales logits per-token -- order-preserving).  Extracted with a
      DVE diag-mask multiply + row reduce (the TensorMaskReduce ISA op
      costs ~1us of DVE SEQ per call in the model -- avoided).
    - The lo slab of each group loads AFTER the hi slab, and the lo-only
      logits stream is emitted after the hi streams, so the exposed tail
      when DMA runs dry is just the last lo stream + softmax (~3us).
      Gram/diag/rsqrt for the last group overlap the lo stream on DVE.
    - inv_norm = rsqrt(ssq) on DVE only (Quake bit-trick seed + 2 Newton
      steps, batched per 2 groups; no ACT table switch).
    - ScalarE Exp reads PSUM logits directly with per-partition
      scale=inv_norm and fused row-accumulate -> softmax numerator +
      denominator in one op; VectorE reciprocal + tensor_scalar give the
      softmax; VectorE max/max_index give the top-8 values and indices
      (descending, distinct indices on ties, matching jax top_k).
    - DMA queues split: hi loads + half the outputs on SP, lo loads +
      proto tables + rest on ACT (both are HWDGE engines) so neither
      sequencer serializes the stream.  Outputs staged [128, 16*8]
      partition-major, flushed in 4 stages; host re-permutes.

Engine budget (cost model, per core): DMA ~47us (bf16x2; the 16 MiB h
payload is the memory roofline) / ~36us (f16e5), PE ~35us, DVE ~22us,
ACT ~5us.  No engine does a second full-data pass.
"""

from contextlib import ExitStack

import numpy as np

import concourse.bass as bass
import concourse.bacc as bacc
import concourse.mybir as mybir
import concourse.tile as tile

N_CORES = 8
T_FULL = 16384
D = 2048
E = 64
K = 8
P = 128
T_CORE = T_FULL // N_CORES   # 2048
N_CHUNKS = D // P            # 16
N_TILES = T_CORE // P        # 16
GRP = 2                      # tiles per group
N_GROUPS = N_TILES // GRP    # 8
TG = GRP * P                 # 256 tokens per group

F32 = mybir.dt.float32
U32 = mybir.dt.uint32
BF16 = mybir.dt.bfloat16
F16 = mybir.dt.float16
F8E5 = mybir.dt.float8e5

# Limb scheme: "bf16x2" = bf16 hi + bf16 lo (16 MiB/core, exact-class);
# "f16e5" = fp16 hi + fp8e5m2 lo (12 MiB/core, ~2 near-tie set swaps).
SCHEME = "bf16x2"

_SCHEMES = {
    "bf16x2": (BF16, BF16),
    "f16e5": (F16, F8E5),
}


def build_program(scheme=None):
    global SCHEME
    if scheme is not None:
        SCHEME = scheme
    hi_dt, lo_dt = _SCHEMES[SCHEME]
    lo_bytes = 1 if lo_dt == F8E5 else 2
    # 1-byte lo slabs span 2 groups so DMA runs stay >= 512B.
    LO_SPAN = 2 if lo_bytes == 1 else 1

    nc = bacc.Bacc(
        "TRN2", target_bir_lowering=False, debug=False, num_devices=N_CORES
    )
    hiT_d = nc.dram_tensor("hiT", [D, T_CORE], hi_dt, kind="ExternalInput").ap()
    loT_d = nc.dram_tensor("loT", [D, T_CORE], lo_dt, kind="ExternalInput").ap()
    pth_d = nc.dram_tensor("pth", [P, N_CHUNKS * E], hi_dt, kind="ExternalInput").ap()
    ptl_d = nc.dram_tensor("ptl", [P, N_CHUNKS * E], hi_dt, kind="ExternalInput").ap()
    pt8_d = nc.dram_tensor("pt8", [P, N_CHUNKS * E], lo_dt, kind="ExternalInput").ap()
    dm_d = nc.dram_tensor("dmask", [P, GRP * P], F32, kind="ExternalInput").ap()
    # single combined output: [:, 0, :] = weights (f32 bits), [:, 1, :] = idx
    oc_d = nc.dram_tensor(
        "out_c", [P, 2, N_TILES * K], U32, kind="ExternalOutput"
    ).ap()

    # d = c*128 + p  ->  [p][c][t]
    hi_v = hiT_d.rearrange("(c p) t -> p c t", p=P)
    lo_v = loT_d.rearrange("(c p) t -> p c t", p=P)

    with tile.TileContext(nc) as tc, ExitStack() as ctx:
        singles = ctx.enter_context(tc.tile_pool(name="singles", bufs=1))
        hi_pool = ctx.enter_context(tc.tile_pool(name="hiq", bufs=6))
        # lo slabs feed only the cheap lo stream; small buf count paces their
        # DMA issue so they don't displace hi slabs in the transfer FIFO.
        lo_pool = ctx.enter_context(
            tc.tile_pool(name="loq", bufs=2 if LO_SPAN == 2 else 3)
        )
        psL_pool = ctx.enter_context(
            tc.tile_pool(name="psL", bufs=3, space=bass.MemorySpace.PSUM)
        )
        psG_pool = ctx.enter_context(
            tc.tile_pool(name="psG", bufs=2, space=bass.MemorySpace.PSUM)
        )
        mr_pool = ctx.enter_context(tc.tile_pool(name="mr", bufs=2))
        small = ctx.enter_context(tc.tile_pool(name="small", bufs=4))

        pth_sb = singles.tile([P, N_CHUNKS * E], hi_dt)
        ptl_sb = singles.tile([P, N_CHUNKS * E], hi_dt)
        pt8_sb = singles.tile([P, N_CHUNKS * E], lo_dt)
        dm_sb = singles.tile([P, GRP * P], F32)

        c_stage = singles.tile([P, 2, N_TILES * K], U32)
        ssq_all = singles.tile([P, N_TILES], F32)
        inv_all = singles.tile([P, N_TILES], F32)
        rs_t1 = singles.tile([P, N_TILES], F32)
        rs_t2 = singles.tile([P, N_TILES], F32)

        def rsqrt_batch(g0, gw, steps=2):
            """inv_all[:, g0:g0+gw] = rsqrt(ssq_all[...]) on DVE only:
            Quake bit-trick seed + Newton steps (no ACT table switch).
            Seed err <= 1.7e-3 -> 4.3e-6 after one step -- already well
            under the fp32-class tolerance; it only scales logits
            per-token, so top-k order is unaffected."""
            xs = ssq_all[:, g0 : g0 + gw]
            ys = inv_all[:, g0 : g0 + gw]
            t1 = rs_t1[:, g0 : g0 + gw]
            t2 = rs_t2[:, g0 : g0 + gw]
            xu = xs.bitcast(U32)
            yu = ys.bitcast(U32)
            nc.vector.tensor_scalar(
                yu, xu, 1, 0xFFFFFFFF,
                op0=mybir.AluOpType.logical_shift_right,
                op1=mybir.AluOpType.bitwise_xor,
            )
            nc.vector.tensor_scalar(
                yu, yu, 0xFFFFFFFF - 0x5F3759DF, None,
                op0=mybir.AluOpType.subtract,
            )
            for _ in range(steps):
                nc.vector.tensor_mul(t1, xs, ys)
                nc.vector.tensor_mul(t2, t1, ys)
                nc.vector.tensor_scalar(
                    t2, t2, -0.5, 1.5,
                    op0=mybir.AluOpType.mult, op1=mybir.AluOpType.add,
                )
                nc.vector.tensor_mul(ys, ys, t2)

        # --- DMA emission -------------------------------------------------
        # hi on SP, lo + tables on ACT (both HWDGE-capable); group 0's hi
        # is split in two so the PE starts after ~0.7 MiB instead of 1 MiB.
        nc.sync.dma_start(pth_sb[:], pth_d[:])
        nc.scalar.dma_start(ptl_sb[:], ptl_d[:])
        nc.scalar.dma_start(pt8_sb[:], pt8_d[:])
        nc.scalar.dma_start(dm_sb[:], dm_d[:])

        hi_slabs = {}  # g -> list of (c0, tile)
        lo_slabs = {}  # pair-group pg -> tile covering tokens [pg*LO_SPAN*TG, ...)

        def load_group(g):
            t0 = g * TG
            splits = [2, 6, 8] if g == 0 else [N_CHUNKS]
            subs, c0 = [], 0
            for nch in splits:
                ht = hi_pool.tile([P, nch, TG], hi_dt, tag="hiq")
                nc.sync.dma_start(ht[:], hi_v[:, c0 : c0 + nch, t0 : t0 + TG])
                subs.append((c0, nch, ht))
                c0 += nch
            hi_slabs[g] = subs
            if g % LO_SPAN == 0:
                span = LO_SPAN * TG
                lt = lo_pool.tile([P, N_CHUNKS, span], lo_dt, tag="loq")
                nc.scalar.dma_start(lt[:], lo_v[:, :, t0 : t0 + span])
                lo_slabs[g // LO_SPAN] = lt

        def hi_win(g, c, k):
            for c0, nch, ht in hi_slabs[g]:
                if c0 <= c < c0 + nch:
                    return ht[:, c - c0, k * P : (k + 1) * P]
            raise AssertionError

        def lo_win(g, c, k):
            lt = lo_slabs[g // LO_SPAN]
            off = (g % LO_SPAN) * TG
            return lt[:, c, off + k * P : off + (k + 1) * P]

        def softmax_group(g, pl):
            for k in range(GRP):
                t = g * GRP + k
                probs = small.tile([P, E], F32, tag="probs")
                den = small.tile([P, 1], F32, tag="den")
                nc.scalar.activation(
                    probs[:],
                    pl[:, k * E : (k + 1) * E],
                    mybir.ActivationFunctionType.Exp,
                    scale=inv_all[:, t : t + 1],
                    accum_out=den[:],
                )
                # top-8 on the undivided probs (same order as the softmax --
                # den > 0 is constant per token), then scale just the 8
                # survivors: keeps reciprocal/multiply off the exp->max->
                # max_index critical path in the kernel tail.
                rden = small.tile([P, 1], F32, tag="rden")
                nc.vector.reciprocal(rden[:], den[:])
                t8p = small.tile([P, K], F32, tag="t8p")
                nc.vector.max(out=t8p[:], in_=probs[:])
                nc.vector.tensor_scalar_mul(
                    c_stage[:, 0, t * K : (t + 1) * K].bitcast(F32),
                    t8p[:], rden[:],
                )
                nc.vector.max_index(
                    out=c_stage[:, 1, t * K : (t + 1) * K],
                    in_max=t8p[:],
                    in_values=probs[:],
                )
            # flush finished quarters of the output staging
            if g % 2 == 1 and g < N_GROUPS - 1:
                q0 = (g - 1) * GRP * K
                q1 = (g + 1) * GRP * K
                eng = nc.sync if (g // 2) % 2 == 0 else nc.scalar
                eng.dma_start(oc_d[:, :, q0:q1], c_stage[:, :, q0:q1])

        AHEAD = 3
        for g in range(min(AHEAD, N_GROUPS)):
            load_group(g)

        psL_prev = None
        for g in range(N_GROUPS):
            if g + AHEAD < N_GROUPS:
                load_group(g + AHEAD)

            psL = psL_pool.tile([P, GRP * E], F32, tag="psL")
            psG = psG_pool.tile([P, GRP * P], F32, tag="psG")
            # Gram stream FIRST: psG finishes ~2us before the A/B streams,
            # so diag+rsqrt on DVE complete while the PE still runs A/B/lo
            # and the softmax never waits on inv_norm.
            for c in range(N_CHUNKS):
                for k in range(GRP):
                    hw = hi_win(g, c, k)
                    nc.tensor.matmul(
                        psG[:, k * P : (k + 1) * P],
                        lhsT=hw, rhs=hw,
                        start=c == 0 and k == 0,
                        stop=c == N_CHUNKS - 1 and k == GRP - 1,
                    )
            # hi logits streams vs pt_hi/pt_lo.
            for c in range(N_CHUNKS):
                for k in range(GRP):
                    hw = hi_win(g, c, k)
                    nc.tensor.matmul(
                        psL[:, k * E : (k + 1) * E],
                        lhsT=hw, rhs=pth_sb[:, c * E : (c + 1) * E],
                        start=c == 0 and k == 0, stop=False,
                    )
                    nc.tensor.matmul(
                        psL[:, k * E : (k + 1) * E],
                        lhsT=hw, rhs=ptl_sb[:, c * E : (c + 1) * E],
                        start=False, stop=False,
                    )
            # softmax of the PREVIOUS group first: its inputs are long done,
            # so it never blocks the DVE queue ahead of this group's
            # diag/rsqrt (which wait on psG).
            if psL_prev is not None:
                softmax_group(g - 1, psL_prev)

            # ||h_t||^2 = Gram diagonal (diag-mask multiply + row-sum) and
            # rsqrt -- emitted BEFORE the lo stream: they only need psG
            # (hi data), so DVE computes inv while the PE chews the lo
            # stream, and the group's softmax can fire the moment the lo
            # stream finishes.
            mr = mr_pool.tile([P, GRP, P], F32, tag="mr")
            nc.vector.tensor_tensor(
                mr[:].rearrange("p g q -> p (g q)"), psG[:], dm_sb[:],
                op=mybir.AluOpType.mult,
            )
            nc.vector.tensor_reduce(
                ssq_all[:, g * GRP : (g + 1) * GRP], mr[:],
                axis=mybir.AxisListType.X, op=mybir.AluOpType.add,
            )
            rsqrt_batch(g * GRP, GRP, steps=1)

            # lo stream last: the only PE work gated on the lo DMA.
            for c in range(N_CHUNKS):
                for k in range(GRP):
                    nc.tensor.matmul(
                        psL[:, k * E : (k + 1) * E],
                        lhsT=lo_win(g, c, k),
                        rhs=pt8_sb[:, c * E : (c + 1) * E],
                        start=False,
                        stop=c == N_CHUNKS - 1 and k == GRP - 1,
                    )

            psL_prev = psL

        softmax_group(N_GROUPS - 1, psL_prev)
        q0 = (N_GROUPS - 2) * GRP * K
        nc.sync.dma_start(oc_d[:, :, q0:], c_stage[:, :, q0:])

    nc.compile()
    return nc


_CACHE = {}


def _get_program():
    if "nc" not in _CACHE:
        _CACHE["nc"] = build_program()
    return _CACHE["nc"]


def _np_dt(dt):
    import ml_dtypes

    return {
        BF16: np.dtype(ml_dtypes.bfloat16),
        F16: np.dtype(np.float16),
        F8E5: np.dtype(ml_dtypes.float8_e5m2),
    }[dt]


def make_inputs_for_cores(hidden_states, proto):
    hi_dt, lo_dt = _SCHEMES[SCHEME]
    nhi, nlo = _np_dt(hi_dt), _np_dt(lo_dt)

    h = np.asarray(hidden_states, dtype=np.float32)
    p = np.asarray(proto, dtype=np.float32)
    assert h.shape == (T_FULL, D) and p.shape == (E, D)

    hi = h.astype(nhi)
    lo = (h - hi.astype(np.float32)).astype(nlo)

    norm = np.linalg.norm(p, axis=1, keepdims=True)
    pn = (p / np.maximum(norm, 1e-12)).astype(np.float32)
    # Scale the proto tables by 32: unit-norm rows have ~0.02 components,
    # whose fp16 limb residuals (~1e-5) are all DENORMAL in fp16 and the PE
    # flushes them to zero on hardware.  32x makes both limbs normal; the
    # logits come out 32x, and the diag mask (32^2) folds the descale into
    # inv_norm = rsqrt(32^2 * ssq) = rsqrt(ssq)/32 for free.
    pn32 = 32.0 * pn
    p_hi = pn32.astype(nhi)
    p_lo = (pn32 - p_hi.astype(np.float32)).astype(nhi)
    p_8 = pn32.astype(nlo)

    def pt_layout(pl):  # [E, D] -> [P, N_CHUNKS*E]: pt[p_, c*64+e] = pl[e, c*128+p_]
        return np.ascontiguousarray(
            pl.T.reshape(N_CHUNKS, P, E).transpose(1, 0, 2)
        ).reshape(P, N_CHUNKS * E)

    pth = pt_layout(p_hi)
    ptl = pt_layout(p_lo)
    pt8 = pt_layout(p_8)
    dmask = np.tile(1024.0 * np.eye(P, dtype=np.float32), (1, GRP))

    ins = []
    for c in range(N_CORES):
        sl = slice(c * T_CORE, (c + 1) * T_CORE)
        ins.append({
            "hiT": np.ascontiguousarray(hi[sl].T),
            "loT": np.ascontiguousarray(lo[sl].T),
            "pth": pth,
            "ptl": ptl,
            "pt8": pt8,
            "dmask": dmask,
        })
    return ins


def unshard_outputs(results):
    w_parts, i_parts = [], []
    for c in range(N_CORES):
        oc = np.asarray(results[c]["out_c"])
        ws = oc[:, 0, :].view(np.float32)
        ix = oc[:, 1, :]
        w_parts.append(ws.reshape(P, N_TILES, K).transpose(1, 0, 2).reshape(T_CORE, K))
        i_parts.append(
            ix.reshape(P, N_TILES, K)
            .transpose(1, 0, 2)
            .reshape(T_CORE, K)
            .astype(np.int32)
        )
    return np.concatenate(w_parts, 0), np.concatenate(i_parts, 0)


def run_on_hw(hidden_states, proto, trace=False):
    from concourse.bass_utils import run_bass_kernel_spmd

    nc = _get_program()
    in_maps = make_inputs_for_cores(hidden_states, proto)
    res = run_bass_kernel_spmd(
        nc, in_maps, core_ids=list(range(N_CORES)), trace=trace
    )
    _CACHE["last_results"] = res
    return unshard_outputs(res.results)


def kernel(hidden_states, proto):
    return run_on_hw(hidden_states, proto, trace=False)
